# revision 29
# baseline (speedup 1.0000x reference)
"""MLA attention (DeepSeek-style) distributed over 8 TRN2 NeuronCores.

Sharding: core c -> batch b=c//4, head-group/seq-shard g=c%4.
All compute/exchange in bf16 (fp8 fails the 2e-2 budget: weight
quantization error is systematic across positions and does not
average out).

Phase A: down-proj of own 512-pos shard. kv latent + rope + inv-rms
row -> one 8-core shared-output AllGather (b_kv: 4 latent slabs |
rope block with the a_kv row packed into partition 64).
Q path never exchanges latents: q-up runs locally (all 16 heads, own
512 cols, local rmsnorm via a rank-1 PE broadcast), then TWO 8-core
AllToAlls ship finished q heads to their owners (slot pairs {0,1}
then {2,3}; each peer receives exactly its own heads x own cols;
both batches' shards carry duplicate data). This is ~5MB of wire vs
~12MB for a bf16 latent AllGather, and the up-proj never waits on a
collective.
Phase B: kv up-proj (Kt d-major, V row-major) from the gathered
latent; a_kv broadcast via rank-1 PE matmul (the gpsimd queue carries
only collectives, so triggers fire promptly; note the CC queue is
fully serial: ~43us bootstrap op + ~10us handshake per collective).
Attention: flash-style causal with St[kv,q] layout; causal mask
folded into the score-matmul PSUM group; softmax denominators via a
bf16 P_sum vector-accumulate + one ones-matmul per (head, q-chunk);
the reciprocal row is partition-broadcast with a rank-1 PE matmul.
Per-head outputs exchange via 8-core AllToAll; the row-parallel wo
matmul on the own s-shard is interleaved between heads and
accumulated in SBUF.
"""

import numpy as np
import ml_dtypes

import concourse.bass as bass
import concourse.bacc as bacc
import concourse.tile as tile
import concourse.mybir as mybir
from concourse.bass_utils import run_bass_kernel_spmd

BF16 = ml_dtypes.bfloat16

# problem constants (hardcoded per harness rules)
DIM = 2048
N_HEADS = 16
Q_LORA = 1536
KV_LORA = 512
NOPE = 128
ROPE = 64
V_DIM = 128
QK_HD = NOPE + ROPE  # 192
EPS = 1e-6
B, S = 2, 2048
SCALE = QK_HD ** -0.5

NCORES = 8
GROUP = 4               # cores per batch
SSH = S // GROUP        # 512, seq shard
HPC = N_HEADS // GROUP  # 4 heads per core
P = 128
NKT = DIM // P          # 16
NQM = Q_LORA // P       # 12
NKVM = KV_LORA // P     # 4
NCH = S // 512          # 4
BKV_W = NKVM * SSH + SSH   # 2560: kv latent slabs | rope + a_kv block
QSH = 2 * QK_HD            # 384 rows: one A2A shard = 2 head-slots
RG8 = [list(range(NCORES))]

_cache = {}


def _build():
    nc = bacc.Bacc("TRN2", target_bir_lowering=False, debug=False,
                   num_devices=NCORES)
    f32 = mybir.dt.float32
    bf = mybir.dt.bfloat16
    i32 = mybir.dt.int32

    # ---- dram parameters (partition-major packed layouts) ----
    xP = nc.dram_tensor("xP", [P, NKT, SSH], bf, kind="ExternalInput")
    wqaP = nc.dram_tensor("wqaP", [P, NQM, NKT, P], bf, kind="ExternalInput")
    wkvaP = nc.dram_tensor("wkvaP", [P, NKVM, NKT, P], bf,
                           kind="ExternalInput")
    wkpeP = nc.dram_tensor("wkpeP", [P, NKT, ROPE], bf, kind="ExternalInput")
    # q-up weights for ALL 16 heads, slot-pair major (see _prep_inputs)
    wqbP = nc.dram_tensor("wqbP", [P, NQM, N_HEADS * QK_HD], bf,
                          kind="ExternalInput")
    wkvbP = nc.dram_tensor("wkvbP", [P, NKVM, HPC * (NOPE + V_DIM)], bf,
                           kind="ExternalInput")
    # wo.T rows regrouped head-major: [p, h, k, dim] = head k*4+h, vdim p
    woP = nc.dram_tensor("woP", [P, HPC, GROUP, DIM], bf,
                         kind="ExternalInput")
    cos_sh = nc.dram_tensor("cos_sh", [P, SSH], bf, kind="ExternalInput")
    sin_sh = nc.dram_tensor("sin_sh", [P, SSH], bf, kind="ExternalInput")
    perm64 = nc.dram_tensor("perm64", [P, P], bf, kind="ExternalInput")
    wmask = nc.dram_tensor("wmask", [P, SSH], bf, kind="ExternalInput")
    cfg = nc.dram_tensor("cfg", [1, 2], i32, kind="ExternalInput")
    outT = nc.dram_tensor("out", [DIM, SSH], f32, kind="ExternalOutput")

    # ---- internal dram ----
    b_kv = nc.dram_tensor("b_kv", [P, BKV_W], bf)
    g_kv = nc.dram_tensor("g_kv", [NCORES * P, BKV_W], bf,
                          addr_space="Shared")
    # q head exchange: shard j = peer j's 2 head-slots (x192 rows), my cols
    b_q1 = nc.dram_tensor("b_q1", [NCORES * QSH, SSH], bf)
    o_q1 = nc.dram_tensor("o_q1", [NCORES * QSH, SSH], bf)
    b_q2 = nc.dram_tensor("b_q2", [NCORES * QSH, SSH], bf)
    o_q2 = nc.dram_tensor("o_q2", [NCORES * QSH, SSH], bf)
    b_o = [nc.dram_tensor(f"b_o{h}", [NCORES * P, SSH], bf)
           for h in range(HPC)]
    o_r = [nc.dram_tensor(f"o_r{h}", [NCORES * P, SSH], bf)
           for h in range(HPC)]

    with tile.TileContext(nc) as tc:
        with (
            tc.tile_pool(name="persist", bufs=1) as persist,
            tc.tile_pool(name="attn", bufs=1) as attn_pool,
            tc.tile_pool(name="wts", bufs=1) as wts,
        ):
            # constants first (engine-local, no DMA)
            ones_b = persist.tile([P, 1], bf)
            nc.vector.memset(ones_b, 1.0)
            ones1 = persist.tile([1, P], bf)
            nc.vector.memset(ones1, 1.0)
            eps_sb = persist.tile([1, 1], f32)
            nc.vector.memset(eps_sb, EPS)
            cfg_sb = persist.tile([1, 2], i32)
            nc.scalar.dma_start(out=cfg_sb, in_=cfg[:])

            # early small tables needed by phase A (scalar queue)
            perm_sb = persist.tile([P, P], bf)
            nc.scalar.dma_start(out=perm_sb, in_=perm64[:])
            cos_sh_sb = persist.tile([P, SSH], bf)
            nc.scalar.dma_start(out=cos_sh_sb, in_=cos_sh[:])
            sin_sh_sb = persist.tile([P, SSH], bf)
            nc.scalar.dma_start(out=sin_sh_sb, in_=sin_sh[:])
            # weights for later phases, in order of first use
            wkvb = wts.tile([P, NKVM, HPC * (NOPE + V_DIM)], bf)
            nc.scalar.dma_start(out=wkvb, in_=wkvbP[:])
            mask_sb = persist.tile([P, SSH], bf)
            nc.scalar.dma_start(out=mask_sb, in_=wmask[:])

            # attention-phase persistent tiles
            qt_nope = [attn_pool.tile([P, S], bf, tag=f"qtn{h}",
                                      name=f"qt_nope{h}") for h in range(HPC)]
            qt_pe = [attn_pool.tile([P, S], bf, tag=f"qtp{h}",
                                    name=f"qt_pe{h}")
                     for h in range(HPC // 2)]
            kt_nope = [attn_pool.tile([P, S], bf, tag=f"ktn{h}",
                                      name=f"kt_nope{h}") for h in range(HPC)]
            v_all = attn_pool.tile([P, S // P, HPC * V_DIM], bf)
            kpe_dup = attn_pool.tile([P, NCH, SSH], bf)

            # ======== Phase A: down-proj + local q-up ========
            up_lat_cm = tc.tile_pool(name="up_lat", bufs=1)
            up_lat = up_lat_cm.__enter__()
            # q latent slabs + aq broadcast live across both sub-phases
            q_lat = up_lat.tile([P, NQM, SSH], bf, name="q_lat")
            aq_sb = up_lat.tile([P, SSH], bf, name="aq_sb")
            with (
                tc.tile_pool(name="pa", bufs=3) as pa,
                tc.tile_pool(name="pa_x", bufs=1) as pa_x,
                tc.tile_pool(name="pa_out", bufs=3) as pa_out,
                tc.tile_pool(name="pa_ps", bufs=2, space="PSUM") as pa_ps,
                tc.tile_pool(name="pa_st", bufs=1, space="PSUM") as pa_st,
            ):
                # x in quarter tiles so the first matmul starts early
                xq = []
                for i in range(4):
                    t = pa_x.tile([P, 4, SSH], bf, name=f"x{i}")
                    nc.sync.dma_start(out=t, in_=xP[:, 4 * i:4 * i + 4, :])
                    xq.append(t)

                q_stat = pa_st.tile([1, SSH], f32)
                kv_stat = pa_st.tile([1, SSH], f32)

                def down_slab(wP_m, mrows, dst, stat_ps, stat_first,
                              stat_last, ev_tag="ev", sbuf_out=None):
                    slab = pa.tile([P, NKT, mrows], bf, tag="slab")
                    nc.sync.dma_start(out=slab, in_=wP_m)
                    ps = pa_ps.tile([P, SSH], f32, tag="dps")
                    for k in range(NKT):
                        nc.tensor.matmul(ps[:mrows, :], slab[:, k, :],
                                         xq[k // 4][:, k % 4, :],
                                         start=(k == 0), stop=(k == NKT - 1))
                    if sbuf_out is not None:
                        ev = sbuf_out
                        nc.vector.tensor_copy(ev, ps[:mrows, :])
                    else:
                        ev = pa_out.tile([P, SSH], bf, tag=ev_tag)
                        nc.vector.tensor_copy(ev[:mrows, :], ps[:mrows, :])
                        if dst is not None:
                            nc.sync.dma_start(out=dst, in_=ev[:mrows, :])
                    if stat_ps is not None:
                        sq = pa.tile([P, SSH], bf, tag="sq")
                        nc.scalar.square(sq[:mrows, :], ps[:mrows, :])
                        nc.tensor.matmul(stat_ps, ones_b[:mrows, :],
                                         sq[:mrows, :], start=stat_first,
                                         stop=stat_last)
                    return ev

                def stat_row(stat, n):
                    # 1/rms row in bf16
                    tmp = pa.tile([1, SSH], f32, tag="srt")
                    nc.scalar.activation(tmp, stat,
                                         mybir.ActivationFunctionType.Sqrt,
                                         bias=eps_sb[0:1, 0:1], scale=1.0 / n)
                    rcp = pa.tile([1, SSH], f32, tag="rcp")
                    nc.vector.reciprocal_approx_fast(rcp, tmp)
                    rcb = pa.tile([1, SSH], bf, tag="rcb")
                    nc.vector.tensor_copy(rcb, rcp)
                    return rcb

                # ---- kv first (AG_kv rides the CC bootstrap) ----
                kpe_ev = down_slab(wkpeP[:], ROPE, None, None,
                                   False, False, ev_tag="kpe_ev")
                for m in range(NKVM):
                    down_slab(wkvaP[:, m], P,
                              b_kv[:, m * SSH:(m + 1) * SSH], kv_stat,
                              m == 0, m == NKVM - 1)
                # k rope on the eviction
                xs_ps = pa_ps.tile([ROPE, SSH], f32, tag="xs")
                nc.tensor.matmul(xs_ps, perm_sb[:ROPE, :ROPE],
                                 kpe_ev[:ROPE, :])
                y0 = pa.tile([ROPE, SSH], bf, tag="ry0")
                nc.vector.tensor_mul(y0, kpe_ev[:ROPE, :],
                                     cos_sh_sb[:ROPE, :])
                y1 = pa.tile([ROPE, SSH], bf, tag="ry1")
                nc.vector.tensor_mul(y1, xs_ps, sin_sh_sb[:ROPE, :])
                yr = pa.tile([ROPE, SSH], bf, tag="ryr")
                nc.vector.tensor_add(yr, y0, y1)
                rope_col = NKVM * SSH
                nc.sync.dma_start(
                    out=b_kv[0:ROPE, rope_col:rope_col + SSH], in_=yr)
                kv_rcb = stat_row(kv_stat, KV_LORA)
                nc.sync.dma_start(
                    out=b_kv[ROPE:ROPE + 1, rope_col:rope_col + SSH],
                    in_=kv_rcb)
                nc.gpsimd.collective_compute(
                    "AllGather", mybir.AluOpType.bypass, replica_groups=RG8,
                    ins=[b_kv[:]], outs=[g_kv[:]])

                # per-core row bases for gather/A2A reads
                r0 = nc.alloc_registers()
                nc.regs_load(r0, cfg_sb[0:1, 0:1])
                rb_base = nc.snap(r0, donate=True, min_val=0,
                                  max_val=(NCORES - GROUP) * P)
                r1 = nc.alloc_registers()
                nc.regs_load(r1, cfg_sb[0:1, 1:2])
                rq_base = nc.snap(r1, donate=True, min_val=0,
                                  max_val=(NCORES - GROUP) * QSH)

                # ---- q down-proj (latent kept local in SBUF) ----
                for m in range(NQM):
                    down_slab(wqaP[:, m], P, None, q_stat,
                              m == 0, m == NQM - 1,
                              sbuf_out=q_lat[:, m, :])
                q_rcb = stat_row(q_stat, Q_LORA)
                bps = pa_ps.tile([P, SSH], f32, tag="bc")
                nc.tensor.matmul(bps, ones1, q_rcb)
                nc.vector.tensor_copy(aq_sb, bps)

            # ======== Phase B: local q-up, A2A_q, kv-up ========
            with (
                tc.tile_pool(name="qup", bufs=3) as qup,
                tc.tile_pool(name="wqp", bufs=3) as wq_pool,
                tc.tile_pool(name="up_ps", bufs=3, space="PSUM") as up_ps,
                tc.tile_pool(name="pe_ps", bufs=2, space="PSUM") as pe_ps,
                tc.tile_pool(name="bc_ps", bufs=1, space="PSUM") as bc_ps,
            ):
                # kv gather loads on the gpsimd queue: it blocks on the
                # AG_kv collective instruction anyway, so these issue the
                # moment the gather lands, without convoying other queues
                kv_lat = up_lat.tile([P, NCH, NKVM, SSH], bf)
                akv_row = up_lat.tile([1, NCH, SSH], bf)
                for r in range(NCH):
                    row = bass.ds(rb_base + r * P, P)
                    nc.gpsimd.dma_start(
                        out=kv_lat[:, r],
                        in_=g_kv[row, 0:NKVM * SSH].rearrange(
                            "p (k s) -> p k s", s=SSH))
                    nc.gpsimd.dma_start(
                        out=kpe_dup[0:ROPE, r, :],
                        in_=g_kv[bass.ds(rb_base + r * P, ROPE),
                                 NKVM * SSH:])
                    nc.gpsimd.dma_start(
                        out=kpe_dup[ROPE:P, r, :],
                        in_=g_kv[bass.ds(rb_base + r * P, ROPE),
                                 NKVM * SSH:])
                    nc.gpsimd.dma_start(
                        out=akv_row[:, r, :],
                        in_=g_kv[bass.ds(rb_base + r * P + ROPE, 1),
                                 NKVM * SSH:])

                # assemble my heads from a finished q A2A (gpsimd queue:
                # sits right behind that A2A's blocking instruction)
                def qt_assemble(hp, oq):
                    for hb in range(2):
                        h = hp * 2 + hb
                        for k in range(GROUP):
                            src = bass.ds(rq_base + k * QSH + hb * QK_HD, P)
                            nc.gpsimd.dma_start(
                                out=qt_nope[h][:, k * SSH:(k + 1) * SSH],
                                in_=oq[src, :])
                    for k in range(GROUP):
                        for hb in range(2):
                            src = bass.ds(
                                rq_base + k * QSH + hb * QK_HD + P, ROPE)
                            nc.gpsimd.dma_start(
                                out=qt_pe[hp][hb * ROPE:(hb + 1) * ROPE,
                                              k * SSH:(k + 1) * SSH],
                                in_=oq[src, :])

                # ---- local q-up: all 16 heads, own cols, pair-major ----
                # pair pi = (half, owner): heads owner*4 + half*2 + {0,1};
                # wqb pair block: [nopeA 128 | nopeB 128 | peA 64 | peB 64]
                for pi in range(N_HEADS // 2):
                    half, owner = pi // 4, pi % 4
                    wqb = wq_pool.tile([P, NQM, QSH], bf, tag="wqb",
                                       name=f"wqb{pi}")
                    nc.scalar.dma_start(
                        out=wqb, in_=wqbP[:, :, pi * QSH:(pi + 1) * QSH])
                    evs = []
                    for hb in range(2):
                        ps = up_ps.tile([P, SSH], f32, tag="qps")
                        for k in range(NQM):
                            nc.tensor.matmul(
                                ps, wqb[:, k, hb * P:(hb + 1) * P],
                                q_lat[:, k, :], start=(k == 0),
                                stop=(k == NQM - 1))
                        ev = qup.tile([P, SSH], bf, tag=f"qev{hb}")
                        nc.vector.tensor_mul(ev, ps, aq_sb)
                        evs.append(ev)
                    pp = pe_ps.tile([P, SSH], f32, tag="qpp")
                    for k in range(NQM):
                        nc.tensor.matmul(
                            pp, wqb[:, k, 2 * P:QSH],
                            q_lat[:, k, :], start=(k == 0),
                            stop=(k == NQM - 1))
                    pe_s = qup.tile([P, SSH], bf, tag="pes")
                    nc.vector.tensor_mul(pe_s, pp, aq_sb)
                    xs = pe_ps.tile([P, SSH], f32, tag="qpx")
                    nc.tensor.matmul(xs, perm_sb, pe_s)
                    pr = qup.tile([P, SSH], bf, tag="per")
                    nc.vector.tensor_mul(pr, pe_s, cos_sh_sb)
                    t1 = qup.tile([P, SSH], bf, tag="pet")
                    nc.vector.tensor_mul(t1, xs, sin_sh_sb)
                    nc.vector.tensor_add(pr, pr, t1)
                    bq = b_q1 if half == 0 else b_q2
                    for db in (0, GROUP):     # duplicate for both batches
                        base = (owner + db) * QSH
                        nc.sync.dma_start(out=bq[base:base + P, :],
                                          in_=evs[0])
                        nc.sync.dma_start(
                            out=bq[base + P:base + P + ROPE, :],
                            in_=pr[0:ROPE, :])
                        nc.sync.dma_start(
                            out=bq[base + QK_HD:base + QK_HD + P, :],
                            in_=evs[1])
                        nc.sync.dma_start(
                            out=bq[base + QK_HD + P:base + QSH, :],
                            in_=pr[ROPE:, :])
                    if pi == 3:
                        nc.gpsimd.collective_compute(
                            "AllToAll", mybir.AluOpType.bypass,
                            replica_groups=RG8, ins=[b_q1[:]],
                            outs=[o_q1[:]])
                        qt_assemble(0, o_q1)
                    if pi == 7:
                        nc.gpsimd.collective_compute(
                            "AllToAll", mybir.AluOpType.bypass,
                            replica_groups=RG8, ins=[b_q2[:]],
                            outs=[o_q2[:]])
                        qt_assemble(1, o_q2)

                # ---- kv up-proj (prescale by broadcast a_kv) ----
                for r in range(NCH):
                    bps = bc_ps.tile([P, SSH], f32, tag="bc")
                    nc.tensor.matmul(bps, ones1, akv_row[:, r, :])
                    akv = qup.tile([P, SSH], bf, tag="akv")
                    nc.vector.tensor_copy(akv, bps)
                    for k in range(NKVM):
                        nc.vector.tensor_mul(kv_lat[:, r, k, :],
                                             kv_lat[:, r, k, :], akv)
                for c in range(NCH):
                    for h in range(HPC):
                        ps = up_ps.tile([P, SSH], f32, tag="qps")
                        for k in range(NKVM):
                            nc.tensor.matmul(
                                ps, wkvb[:, k, h * NOPE:(h + 1) * NOPE],
                                kv_lat[:, c, k, :], start=(k == 0),
                                stop=(k == NKVM - 1))
                        nc.vector.tensor_copy(
                            kt_nope[h][:, c * SSH:(c + 1) * SSH], ps)
                for sb in range(S // P):
                    c, part = sb // 4, sb % 4
                    ps = up_ps.tile([P, HPC * V_DIM], f32, tag="qps")
                    for k in range(NKVM):
                        nc.tensor.matmul(
                            ps, kv_lat[:, c, k, part * P:(part + 1) * P],
                            wkvb[:, k, HPC * NOPE:], start=(k == 0),
                            stop=(k == NKVM - 1))
                    nc.vector.tensor_copy(v_all[:, sb, :], ps)


            up_lat_cm.__exit__(None, None, None)

            # ===== attention + per-head A2A + interleaved wo =====
            with (
                tc.tile_pool(name="at", bufs=3) as at,
                tc.tile_pool(name="atp", bufs=16) as atp,
                tc.tile_pool(name="at_ps", bufs=2) as at_psum_sb,
                tc.tile_pool(name="at_rl", bufs=2) as at_rl,
                tc.tile_pool(name="wo_rhs", bufs=3) as wo_rhs,
                tc.tile_pool(name="wo_acc", bufs=1) as wo_acc,
                tc.tile_pool(name="wo_w", bufs=3) as wo_w,
                tc.tile_pool(name="wo_ev", bufs=3) as wo_ev,
                tc.tile_pool(name="st_ps", bufs=2, space="PSUM") as st_ps,
                tc.tile_pool(name="ot_ps", bufs=2, space="PSUM") as ot_ps,
                tc.tile_pool(name="l_ps", bufs=1, space="PSUM") as l_ps,
                tc.tile_pool(name="rb_ps", bufs=1, space="PSUM") as rb_ps,
                tc.tile_pool(name="wo_ps", bufs=2, space="PSUM") as wo_ps,
            ):
                acc = wo_acc.tile([P, NKT, SSH], f32)

                def attention_head(h):
                    pend = []        # [(pj, off, first, last, ot, j), ...]
                    pend_ev = None   # (ot, P_sum, qc)

                    def flush2():
                        while pend:
                            pj, off, first, last, ot, j = pend.pop(0)
                            nc.tensor.matmul(
                                ot[:, off:],
                                v_all[:, j, h * V_DIM:(h + 1) * V_DIM],
                                pj[:, off:], start=first, stop=last)

                    def evict():
                        nonlocal pend_ev
                        if pend_ev is None:
                            return
                        ot, psum, qc = pend_ev
                        lt = l_ps.tile([1, SSH], f32, tag="l", name="lt")
                        nc.tensor.matmul(lt, ones_b, psum)
                        rinv = at_rl.tile([1, SSH], f32, tag="ri",
                                          name="rinv")
                        nc.vector.reciprocal_approx_fast(rinv, lt)
                        rinvb = at_rl.tile([1, SSH], bf, tag="rib",
                                           name="rinvb")
                        nc.vector.tensor_copy(rinvb, rinv)
                        rbp = rb_ps.tile([P, SSH], f32, tag="rb",
                                         name="rbp")
                        nc.tensor.matmul(rbp, ones1, rinvb)
                        rlb = at_rl.tile([P, SSH], bf, tag="rlb",
                                         name="rlb")
                        nc.vector.tensor_copy(rlb, rbp)
                        ev = at.tile([P, SSH], bf, tag="oev", name="oev")
                        nc.vector.tensor_mul(ev, ot, rlb)
                        # A2A shard for rank b*4+qc of each batch b
                        nc.sync.dma_start(
                            out=b_o[h][qc * P:(qc + 1) * P, :], in_=ev)
                        nc.sync.dma_start(
                            out=b_o[h][(GROUP + qc) * P:
                                       (GROUP + qc + 1) * P, :], in_=ev)
                        pend_ev = None

                    for qc in range(NCH):
                        nj = qc * 4 + 4
                        ot = ot_ps.tile([P, SSH], f32, tag="ot", name="ot")
                        psum = at_psum_sb.tile([P, SSH], bf, tag="ps",
                                               name="psum")
                        for j in range(nj):
                            d = j - qc * 4
                            off = max(0, d) * P
                            st = st_ps.tile([P, SSH], f32, tag="st",
                                            name="st")
                            nc.tensor.matmul(
                                st[:, off:],
                                kt_nope[h][:, j * P:(j + 1) * P],
                                qt_nope[h][:, qc * SSH + off:(qc + 1) * SSH],
                                start=True, stop=False)
                            lo = (h % 2) * ROPE
                            nc.tensor.matmul(
                                st[:, off:],
                                kpe_dup[lo:lo + ROPE, j // 4,
                                        (j % 4) * P:(j % 4 + 1) * P],
                                qt_pe[h // 2][lo:lo + ROPE,
                                              qc * SSH + off:(qc + 1) * SSH],
                                start=False, stop=True)
                            if j == 1:
                                evict()  # previous qc, off the exp path
                            pj = atp.tile([P, SSH], bf, tag="p", name="pj")
                            nc.scalar.activation(
                                pj[:, off:], st[:, off:],
                                mybir.ActivationFunctionType.Exp)
                            if d >= 0:
                                # multiplicative causal mask (vector)
                                nc.vector.tensor_mul(
                                    pj[:, off:], pj[:, off:],
                                    mask_sb[:, 0:SSH - off])
                            if j == 0:
                                nc.vector.tensor_copy(psum, pj)
                            else:
                                nc.vector.tensor_add(psum[:, off:],
                                                     psum[:, off:],
                                                     pj[:, off:])
                            pend.append((pj, off, j == 0, j == nj - 1,
                                         ot, j))
                        flush2()
                        pend_ev = (ot, psum, qc)
                    evict()
                    nc.gpsimd.collective_compute(
                        "AllToAll", mybir.AluOpType.bypass,
                        replica_groups=RG8, ins=[b_o[h][:]],
                        outs=[o_r[h][:]])
                    # wo weights prefetch on sync (no deps); rhs on gpsimd
                    # right behind the blocking A2A instruction so the
                    # scalar queue (exp) is never convoyed by these waits
                    wslab = wo_w.tile([P, GROUP, DIM], bf, tag="woslab",
                                      name="wslab")
                    nc.sync.dma_start(out=wslab, in_=woP[:, h])
                    rhs = wo_rhs.tile([P, GROUP, SSH], bf, tag="rhs",
                                      name="rhs")
                    for k in range(GROUP):
                        nc.gpsimd.dma_start(
                            out=rhs[:, k, :],
                            in_=o_r[h][bass.ds(rb_base + k * P, P), :])
                    return rhs, wslab

                def wo_pass(h, rhs, wslab):
                    for m in range(NKT):
                        ps = wo_ps.tile([P, SSH], f32, tag="wops",
                                        name="wops")
                        for k in range(GROUP):
                            nc.tensor.matmul(
                                ps, wslab[:, k, m * P:(m + 1) * P],
                                rhs[:, k, :], start=(k == 0),
                                stop=(k == GROUP - 1))
                        if h == 0:
                            nc.vector.tensor_copy(acc[:, m, :], ps)
                        elif h < HPC - 1:
                            nc.vector.tensor_add(acc[:, m, :], ps,
                                                 acc[:, m, :])
                        else:
                            ev = wo_ev.tile([P, SSH], f32, tag="woev",
                                            name="woev")
                            nc.vector.tensor_add(ev, ps, acc[:, m, :])
                            nc.sync.dma_start(out=outT[m * P:(m + 1) * P, :],
                                              in_=ev)

                heads_rhs = {}
                for h in range(HPC):
                    heads_rhs[h] = attention_head(h)
                    if h >= 1:
                        wo_pass(h - 1, *heads_rhs[h - 1])
                wo_pass(HPC - 1, *heads_rhs[HPC - 1])

    nc.compile()
    return nc


def _prep_inputs(x, freqs_cos, freqs_sin, wq_a, q_norm_w, wq_b, wkv_a,
                 kv_norm_w, wkv_b, wo):
    x = np.asarray(x, np.float32)
    freqs_cos = np.asarray(freqs_cos, np.float32)
    freqs_sin = np.asarray(freqs_sin, np.float32)
    wq_a = np.asarray(wq_a, np.float32)
    q_norm_w = np.asarray(q_norm_w, np.float32)
    wq_b = np.asarray(wq_b, np.float32)
    wkv_a = np.asarray(wkv_a, np.float32)
    kv_norm_w = np.asarray(kv_norm_w, np.float32)
    wkv_b = np.asarray(wkv_b, np.float32)
    wo = np.asarray(wo, np.float32)

    wqaT = np.ascontiguousarray(wq_a.T)          # [DIM, Q_LORA]
    wkvaT = np.ascontiguousarray(wkv_a.T)        # [DIM, KV_LORA+ROPE]
    wqaP = np.ascontiguousarray(
        wqaT.reshape(NKT, P, NQM, P).transpose(1, 2, 0, 3)).astype(BF16)
    wkvaP = np.ascontiguousarray(
        wkvaT[:, :KV_LORA].reshape(NKT, P, NKVM, P)
        .transpose(1, 2, 0, 3)).astype(BF16)
    wkpeP = np.ascontiguousarray(
        wkvaT[:, KV_LORA:].reshape(NKT, P, ROPE)
        .transpose(1, 0, 2)).astype(BF16)

    wqb_eff = (wq_b * q_norm_w[None, :]) * SCALE
    wqb_eff = wqb_eff.reshape(N_HEADS, QK_HD, Q_LORA)
    wkvb_eff = wkv_b * kv_norm_w[None, :]
    wkvb_eff = wkvb_eff.reshape(N_HEADS, NOPE + V_DIM, KV_LORA)

    cosT = np.tile(np.repeat(freqs_cos.T, 2, axis=0), (2, 1))  # [128, S]
    sinT = np.tile(np.repeat(freqs_sin.T, 2, axis=0), (2, 1))

    perm64_ = np.zeros((ROPE, ROPE), np.float32)
    for i in range(ROPE // 2):
        perm64_[2 * i + 1, 2 * i] = -1.0  # out[2i]   = -x[2i+1]
        perm64_[2 * i, 2 * i + 1] = 1.0   # out[2i+1] =  x[2i]
    perm = np.zeros((P, P), np.float32)
    perm[:ROPE, :ROPE] = perm64_
    perm[ROPE:, ROPE:] = perm64_
    r = np.arange(P)
    # multiplicative causal mask for diagonal score blocks: [kv r, q t]
    wmask = np.ones((P, SSH), np.float32)
    wmask[:, :P] = np.where(r[:, None] <= r[None, :], 1.0, 0.0)

    # wo.T rows regrouped so pass h contracts head k*4+h for k=0..3
    woT4 = wo.T.reshape(N_HEADS // 4, 4, V_DIM, DIM)  # [k, h, p, D]
    woP = np.ascontiguousarray(woT4.transpose(2, 1, 0, 3)).astype(BF16)

    # q-up weight columns pair-major: pair pi = half*4 + owner covers
    # heads A = owner*4 + half*2, B = A+1; block = [nopeA|nopeB|peA|peB]
    wqb_cols = np.zeros((N_HEADS * QK_HD, Q_LORA), np.float32)
    for pi in range(N_HEADS // 2):
        half, owner = pi // 4, pi % 4
        ha = owner * HPC + half * 2
        base = pi * QSH
        wqb_cols[base:base + NOPE] = wqb_eff[ha, :NOPE]
        wqb_cols[base + NOPE:base + 2 * NOPE] = wqb_eff[ha + 1, :NOPE]
        wqb_cols[base + 2 * NOPE:base + 2 * NOPE + ROPE] = wqb_eff[ha, NOPE:]
        wqb_cols[base + 2 * NOPE + ROPE:base + QSH] = wqb_eff[ha + 1, NOPE:]
    wqbT = wqb_cols.T                                 # [Q_LORA, 16*192]
    wqbP = np.ascontiguousarray(
        wqbT.reshape(NQM, P, N_HEADS * QK_HD).transpose(1, 0, 2)).astype(BF16)

    in_maps = []
    for c in range(NCORES):
        b, g = c // GROUP, c % GROUP
        heads = slice(g * HPC, (g + 1) * HPC)
        xTc = np.ascontiguousarray(x[b].T[:, g * SSH:(g + 1) * SSH])
        xPc = np.ascontiguousarray(
            xTc.reshape(NKT, P, SSH).transpose(1, 0, 2)).astype(BF16)
        wkvbT = np.concatenate(
            [wkvb_eff[heads, :NOPE].reshape(HPC * NOPE, KV_LORA),
             wkvb_eff[heads, NOPE:].reshape(HPC * V_DIM, KV_LORA)],
            axis=0).T                                  # [KV_LORA, 1024]
        wkvbP = np.ascontiguousarray(
            wkvbT.reshape(NKVM, P, HPC * (NOPE + V_DIM))
            .transpose(1, 0, 2)).astype(BF16)
        in_maps.append({
            "xP": xPc,
            "wqaP": wqaP,
            "wkvaP": wkvaP,
            "wkpeP": wkpeP,
            "wqbP": wqbP,
            "wkvbP": wkvbP,
            "woP": woP,
            "cos_sh": np.ascontiguousarray(
                cosT[:, g * SSH:(g + 1) * SSH]).astype(BF16),
            "sin_sh": np.ascontiguousarray(
                sinT[:, g * SSH:(g + 1) * SSH]).astype(BF16),
            "perm64": perm.astype(BF16),
            "wmask": wmask.astype(BF16),
            "cfg": np.array([[b * GROUP * P, b * GROUP * QSH]], np.int32),
        })
    return in_maps


def _run(inputs, trace=False, **kw):
    if "nc" not in _cache:
        _cache["nc"] = _build()
    nc = _cache["nc"]
    in_maps = _prep_inputs(**inputs)
    res = run_bass_kernel_spmd(nc, in_maps, core_ids=list(range(NCORES)),
                               trace=trace, **kw)
    out = np.empty((B, S, DIM), np.float32)
    for c in range(NCORES):
        b, g = c // GROUP, c % GROUP
        out[b, g * SSH:(g + 1) * SSH, :] = res.results[c]["out"].T
    return out, res


def kernel(**inputs):
    out, _ = _run(inputs)
    return out


# revision 32
# speedup vs baseline: 1.0148x; 1.0148x over previous
"""MLA attention (DeepSeek-style) distributed over 8 TRN2 NeuronCores.

Sharding: core c -> batch b=c//4, head-group/seq-shard g=c%4.
All compute/exchange in bf16 (fp8 fails the 2e-2 budget: weight
quantization error is systematic across positions and does not
average out).

Phase A: down-proj of own 512-pos shard. kv latent + rope + inv-rms
row -> one 8-core shared-output AllGather (b_kv: 4 latent slabs |
rope block with the a_kv row packed into partition 64).
Q path never exchanges latents: q-up runs locally (all 16 heads, own
512 cols, local rmsnorm via a rank-1 PE broadcast), then TWO 8-core
AllToAlls ship finished q heads to their owners (slot pairs {0,1}
then {2,3}; each peer receives exactly its own heads x own cols;
both batches' shards carry duplicate data). This is ~5MB of wire vs
~12MB for a bf16 latent AllGather, and the up-proj never waits on a
collective.
Phase B: kv up-proj (Kt d-major, V row-major) from the gathered
latent; a_kv broadcast via rank-1 PE matmul (the gpsimd queue carries
only collectives, so triggers fire promptly; note the CC queue is
fully serial: ~43us bootstrap op + ~10us handshake per collective).
Attention: flash-style causal with St[kv,q] layout; causal mask
folded into the score-matmul PSUM group; softmax denominators via a
bf16 P_sum vector-accumulate + one ones-matmul per (head, q-chunk);
the reciprocal row is partition-broadcast with a rank-1 PE matmul.
Per-head outputs exchange via 8-core AllToAll; the row-parallel wo
matmul on the own s-shard is interleaved between heads and
accumulated in SBUF.
"""

import numpy as np
import ml_dtypes

import concourse.bass as bass
import concourse.bacc as bacc
import concourse.tile as tile
import concourse.mybir as mybir
from concourse.bass_utils import run_bass_kernel_spmd

BF16 = ml_dtypes.bfloat16

# problem constants (hardcoded per harness rules)
DIM = 2048
N_HEADS = 16
Q_LORA = 1536
KV_LORA = 512
NOPE = 128
ROPE = 64
V_DIM = 128
QK_HD = NOPE + ROPE  # 192
EPS = 1e-6
B, S = 2, 2048
SCALE = QK_HD ** -0.5

NCORES = 8
GROUP = 4               # cores per batch
SSH = S // GROUP        # 512, seq shard
HPC = N_HEADS // GROUP  # 4 heads per core
P = 128
NKT = DIM // P          # 16
NQM = Q_LORA // P       # 12
NKVM = KV_LORA // P     # 4
NCH = S // 512          # 4
BKV_W = NKVM * SSH + SSH   # 2560: kv latent slabs | rope + a_kv block
QSH = 2 * QK_HD            # 384 rows: one A2A shard = 2 head-slots
RG8 = [list(range(NCORES))]

_cache = {}


def _build():
    nc = bacc.Bacc("TRN2", target_bir_lowering=False, debug=False,
                   num_devices=NCORES)
    f32 = mybir.dt.float32
    bf = mybir.dt.bfloat16
    i32 = mybir.dt.int32

    # ---- dram parameters (partition-major packed layouts) ----
    xP = nc.dram_tensor("xP", [P, NKT, SSH], bf, kind="ExternalInput")
    wqaP = nc.dram_tensor("wqaP", [P, NQM, NKT, P], bf, kind="ExternalInput")
    wkvaP = nc.dram_tensor("wkvaP", [P, NKVM, NKT, P], bf,
                           kind="ExternalInput")
    wkpeP = nc.dram_tensor("wkpeP", [P, NKT, ROPE], bf, kind="ExternalInput")
    # q-up weights for ALL 16 heads, slot-pair major (see _prep_inputs)
    wqbP = nc.dram_tensor("wqbP", [P, NQM, N_HEADS * QK_HD], bf,
                          kind="ExternalInput")
    wkvbP = nc.dram_tensor("wkvbP", [P, NKVM, HPC * (NOPE + V_DIM)], bf,
                           kind="ExternalInput")
    # wo.T rows regrouped head-major: [p, h, k, dim] = head k*4+h, vdim p
    woP = nc.dram_tensor("woP", [P, HPC, GROUP, DIM], bf,
                         kind="ExternalInput")
    cos_sh = nc.dram_tensor("cos_sh", [P, SSH], bf, kind="ExternalInput")
    sin_sh = nc.dram_tensor("sin_sh", [P, SSH], bf, kind="ExternalInput")
    perm64 = nc.dram_tensor("perm64", [P, P], bf, kind="ExternalInput")
    wmask = nc.dram_tensor("wmask", [P, SSH], bf, kind="ExternalInput")
    cfg = nc.dram_tensor("cfg", [1, 2], i32, kind="ExternalInput")
    outT = nc.dram_tensor("out", [DIM, SSH], f32, kind="ExternalOutput")

    # ---- internal dram ----
    b_kv = nc.dram_tensor("b_kv", [P, BKV_W], bf)
    g_kv = nc.dram_tensor("g_kv", [NCORES * P, BKV_W], bf,
                          addr_space="Shared")
    # q head exchange: shard j = peer j's 2 head-slots (x192 rows), my cols
    b_q1 = nc.dram_tensor("b_q1", [NCORES * QSH, SSH], bf)
    o_q1 = nc.dram_tensor("o_q1", [NCORES * QSH, SSH], bf)
    b_q2 = nc.dram_tensor("b_q2", [NCORES * QSH, SSH], bf)
    o_q2 = nc.dram_tensor("o_q2", [NCORES * QSH, SSH], bf)
    b_o = [nc.dram_tensor(f"b_o{h}", [NCORES * P, SSH], bf)
           for h in range(HPC)]
    o_r = [nc.dram_tensor(f"o_r{h}", [NCORES * P, SSH], bf)
           for h in range(HPC)]

    with tile.TileContext(nc) as tc:
        with (
            tc.tile_pool(name="persist", bufs=1) as persist,
            tc.tile_pool(name="attn", bufs=1) as attn_pool,
            tc.tile_pool(name="wts", bufs=1) as wts,
        ):
            # constants first (engine-local, no DMA)
            ones_b = persist.tile([P, 1], bf)
            nc.vector.memset(ones_b, 1.0)
            ones1 = persist.tile([1, P], bf)
            nc.vector.memset(ones1, 1.0)
            eps_sb = persist.tile([1, 1], f32)
            nc.vector.memset(eps_sb, EPS)
            cfg_sb = persist.tile([1, 2], i32)
            nc.scalar.dma_start(out=cfg_sb, in_=cfg[:])

            # early small tables needed by phase A (scalar queue)
            perm_sb = persist.tile([P, P], bf)
            nc.scalar.dma_start(out=perm_sb, in_=perm64[:])
            cos_sh_sb = persist.tile([P, SSH], bf)
            nc.scalar.dma_start(out=cos_sh_sb, in_=cos_sh[:])
            sin_sh_sb = persist.tile([P, SSH], bf)
            nc.scalar.dma_start(out=sin_sh_sb, in_=sin_sh[:])
            # weights for later phases, in order of first use
            wkvb = wts.tile([P, NKVM, HPC * (NOPE + V_DIM)], bf)
            nc.scalar.dma_start(out=wkvb, in_=wkvbP[:])
            mask_sb = persist.tile([P, SSH], bf)
            nc.scalar.dma_start(out=mask_sb, in_=wmask[:])

            # attention-phase persistent tiles
            qt_nope = [attn_pool.tile([P, S], bf, tag=f"qtn{h}",
                                      name=f"qt_nope{h}") for h in range(HPC)]
            qt_pe = [attn_pool.tile([P, S], bf, tag=f"qtp{h}",
                                    name=f"qt_pe{h}")
                     for h in range(HPC // 2)]
            kt_nope = [attn_pool.tile([P, S], bf, tag=f"ktn{h}",
                                      name=f"kt_nope{h}") for h in range(HPC)]
            v_all = attn_pool.tile([P, S // P, HPC * V_DIM], bf)
            kpe_dup = attn_pool.tile([P, NCH, SSH], bf)

            # ======== Phase A: down-proj + local q-up ========
            up_lat_cm = tc.tile_pool(name="up_lat", bufs=1)
            up_lat = up_lat_cm.__enter__()
            # q latent slabs + aq broadcast live across both sub-phases
            q_lat = up_lat.tile([P, NQM, SSH], bf, name="q_lat")
            aq_sb = up_lat.tile([P, SSH], bf, name="aq_sb")
            with (
                tc.tile_pool(name="pa", bufs=3) as pa,
                tc.tile_pool(name="pa_x", bufs=1) as pa_x,
                tc.tile_pool(name="pa_out", bufs=3) as pa_out,
                tc.tile_pool(name="pa_ps", bufs=2, space="PSUM") as pa_ps,
                tc.tile_pool(name="pa_st", bufs=1, space="PSUM") as pa_st,
            ):
                # first slabs before x so the PE can start ASAP; x in
                # eighth tiles so the first matmul waits on ~384KB only
                pre_kpe = pa.tile([P, NKT, ROPE], bf, tag="slab",
                                  name="pre_kpe")
                nc.sync.dma_start(out=pre_kpe, in_=wkpeP[:])
                pre_kv0 = pa.tile([P, NKT, P], bf, tag="slab",
                                  name="pre_kv0")
                nc.sync.dma_start(out=pre_kv0, in_=wkvaP[:, 0])
                xq = []
                for i in range(8):
                    t = pa_x.tile([P, 2, SSH], bf, name=f"x{i}")
                    nc.sync.dma_start(out=t, in_=xP[:, 2 * i:2 * i + 2, :])
                    xq.append(t)

                q_stat = pa_st.tile([1, SSH], f32)
                kv_stat = pa_st.tile([1, SSH], f32)

                def down_slab(wP_m, mrows, dst, stat_ps, stat_first,
                              stat_last, ev_tag="ev", sbuf_out=None,
                              slab=None):
                    if slab is None:
                        slab = pa.tile([P, NKT, mrows], bf, tag="slab")
                        nc.sync.dma_start(out=slab, in_=wP_m)
                    ps = pa_ps.tile([P, SSH], f32, tag="dps")
                    for k in range(NKT):
                        nc.tensor.matmul(ps[:mrows, :], slab[:, k, :],
                                         xq[k // 2][:, k % 2, :],
                                         start=(k == 0), stop=(k == NKT - 1))
                    if sbuf_out is not None:
                        ev = sbuf_out
                        nc.vector.tensor_copy(ev, ps[:mrows, :])
                    else:
                        ev = pa_out.tile([P, SSH], bf, tag=ev_tag)
                        nc.vector.tensor_copy(ev[:mrows, :], ps[:mrows, :])
                        if dst is not None:
                            nc.sync.dma_start(out=dst, in_=ev[:mrows, :])
                    if stat_ps is not None:
                        sq = pa.tile([P, SSH], bf, tag="sq")
                        nc.scalar.square(sq[:mrows, :], ps[:mrows, :])
                        nc.tensor.matmul(stat_ps, ones_b[:mrows, :],
                                         sq[:mrows, :], start=stat_first,
                                         stop=stat_last)
                    return ev

                def stat_row(stat, n):
                    # 1/rms row in bf16
                    tmp = pa.tile([1, SSH], f32, tag="srt")
                    nc.scalar.activation(tmp, stat,
                                         mybir.ActivationFunctionType.Sqrt,
                                         bias=eps_sb[0:1, 0:1], scale=1.0 / n)
                    rcp = pa.tile([1, SSH], f32, tag="rcp")
                    nc.vector.reciprocal_approx_fast(rcp, tmp)
                    rcb = pa.tile([1, SSH], bf, tag="rcb")
                    nc.vector.tensor_copy(rcb, rcp)
                    return rcb

                # ---- kv first (AG_kv rides the CC bootstrap) ----
                kpe_ev = down_slab(wkpeP[:], ROPE, None, None,
                                   False, False, ev_tag="kpe_ev",
                                   slab=pre_kpe)
                for m in range(NKVM):
                    down_slab(wkvaP[:, m], P,
                              b_kv[:, m * SSH:(m + 1) * SSH], kv_stat,
                              m == 0, m == NKVM - 1,
                              slab=pre_kv0 if m == 0 else None)
                # k rope on the eviction
                xs_ps = pa_ps.tile([ROPE, SSH], f32, tag="xs")
                nc.tensor.matmul(xs_ps, perm_sb[:ROPE, :ROPE],
                                 kpe_ev[:ROPE, :])
                y0 = pa.tile([ROPE, SSH], bf, tag="ry0")
                nc.vector.tensor_mul(y0, kpe_ev[:ROPE, :],
                                     cos_sh_sb[:ROPE, :])
                y1 = pa.tile([ROPE, SSH], bf, tag="ry1")
                nc.vector.tensor_mul(y1, xs_ps, sin_sh_sb[:ROPE, :])
                yr = pa.tile([ROPE, SSH], bf, tag="ryr")
                nc.vector.tensor_add(yr, y0, y1)
                rope_col = NKVM * SSH
                nc.sync.dma_start(
                    out=b_kv[0:ROPE, rope_col:rope_col + SSH], in_=yr)
                kv_rcb = stat_row(kv_stat, KV_LORA)
                nc.sync.dma_start(
                    out=b_kv[ROPE:ROPE + 1, rope_col:rope_col + SSH],
                    in_=kv_rcb)
                nc.gpsimd.collective_compute(
                    "AllGather", mybir.AluOpType.bypass, replica_groups=RG8,
                    ins=[b_kv[:]], outs=[g_kv[:]])

                # per-core row bases for gather/A2A reads
                r0 = nc.alloc_registers()
                nc.regs_load(r0, cfg_sb[0:1, 0:1])
                rb_base = nc.snap(r0, donate=True, min_val=0,
                                  max_val=(NCORES - GROUP) * P)
                r1 = nc.alloc_registers()
                nc.regs_load(r1, cfg_sb[0:1, 1:2])
                rq_base = nc.snap(r1, donate=True, min_val=0,
                                  max_val=(NCORES - GROUP) * QSH)

                # ---- q down-proj (latent kept local in SBUF) ----
                for m in range(NQM):
                    down_slab(wqaP[:, m], P, None, q_stat,
                              m == 0, m == NQM - 1,
                              sbuf_out=q_lat[:, m, :])
                q_rcb = stat_row(q_stat, Q_LORA)
                q_rcb_p = up_lat.tile([1, SSH], bf, name="q_rcb_p")
                nc.vector.tensor_copy(q_rcb_p, q_rcb)

            # ======== Phase B: local q-up, A2A_q, kv-up ========
            with (
                tc.tile_pool(name="qup", bufs=3) as qup,
                tc.tile_pool(name="wqp", bufs=3) as wq_pool,
                tc.tile_pool(name="up_ps", bufs=3, space="PSUM") as up_ps,
                tc.tile_pool(name="pe_ps", bufs=2, space="PSUM") as pe_ps,
                tc.tile_pool(name="bc_ps", bufs=1, space="PSUM") as bc_ps,
            ):
                # kv gather loads on the gpsimd queue: it blocks on the
                # AG_kv collective instruction anyway, so these issue the
                # moment the gather lands, without convoying other queues
                kv_lat = up_lat.tile([P, NCH, NKVM, SSH], bf)
                akv_row = up_lat.tile([1, NCH, SSH], bf)
                for r in range(NCH):
                    row = bass.ds(rb_base + r * P, P)
                    nc.gpsimd.dma_start(
                        out=kv_lat[:, r],
                        in_=g_kv[row, 0:NKVM * SSH].rearrange(
                            "p (k s) -> p k s", s=SSH))
                    nc.gpsimd.dma_start(
                        out=kpe_dup[0:ROPE, r, :],
                        in_=g_kv[bass.ds(rb_base + r * P, ROPE),
                                 NKVM * SSH:])
                    nc.gpsimd.dma_start(
                        out=kpe_dup[ROPE:P, r, :],
                        in_=g_kv[bass.ds(rb_base + r * P, ROPE),
                                 NKVM * SSH:])
                    nc.gpsimd.dma_start(
                        out=akv_row[:, r, :],
                        in_=g_kv[bass.ds(rb_base + r * P + ROPE, 1),
                                 NKVM * SSH:])

                # assemble my heads from a finished q A2A (gpsimd queue:
                # sits right behind that A2A's blocking instruction)
                def qt_assemble(hp, oq):
                    for hb in range(2):
                        h = hp * 2 + hb
                        for k in range(GROUP):
                            src = bass.ds(rq_base + k * QSH + hb * QK_HD, P)
                            nc.gpsimd.dma_start(
                                out=qt_nope[h][:, k * SSH:(k + 1) * SSH],
                                in_=oq[src, :])
                    for k in range(GROUP):
                        for hb in range(2):
                            src = bass.ds(
                                rq_base + k * QSH + hb * QK_HD + P, ROPE)
                            nc.gpsimd.dma_start(
                                out=qt_pe[hp][hb * ROPE:(hb + 1) * ROPE,
                                              k * SSH:(k + 1) * SSH],
                                in_=oq[src, :])

                # ---- local q-up: all 16 heads, own cols, pair-major ----
                # pair pi = (half, owner): heads owner*4 + half*2 + {0,1};
                # wqb pair block: [nopeA 128 | nopeB 128 | peA 64 | peB 64]
                for pi in range(N_HEADS // 2):
                    half, owner = pi // 4, pi % 4
                    wqb = wq_pool.tile([P, NQM, QSH], bf, tag="wqb",
                                       name=f"wqb{pi}")
                    nc.scalar.dma_start(
                        out=wqb, in_=wqbP[:, :, pi * QSH:(pi + 1) * QSH])
                    # matmul groups first (PE never FIFO-blocked on the
                    # rms chain), then the aq broadcast (pi 0), then muls
                    pss = []
                    for hb in range(2):
                        ps = up_ps.tile([P, SSH], f32, tag="qps")
                        for k in range(NQM):
                            nc.tensor.matmul(
                                ps, wqb[:, k, hb * P:(hb + 1) * P],
                                q_lat[:, k, :], start=(k == 0),
                                stop=(k == NQM - 1))
                        pss.append(ps)
                    pp = pe_ps.tile([P, SSH], f32, tag="qpp")
                    for k in range(NQM):
                        nc.tensor.matmul(
                            pp, wqb[:, k, 2 * P:QSH],
                            q_lat[:, k, :], start=(k == 0),
                            stop=(k == NQM - 1))
                    if pi == 0:
                        bps = bc_ps.tile([P, SSH], f32, tag="bc")
                        nc.tensor.matmul(bps, ones1, q_rcb_p)
                        nc.vector.tensor_copy(aq_sb, bps)
                    evs = []
                    for hb in range(2):
                        ev = qup.tile([P, SSH], bf, tag=f"qev{hb}")
                        nc.vector.tensor_mul(ev, pss[hb], aq_sb)
                        evs.append(ev)
                    pe_s = qup.tile([P, SSH], bf, tag="pes")
                    nc.vector.tensor_mul(pe_s, pp, aq_sb)
                    xs = pe_ps.tile([P, SSH], f32, tag="qpx")
                    nc.tensor.matmul(xs, perm_sb, pe_s)
                    pr = qup.tile([P, SSH], bf, tag="per")
                    nc.vector.tensor_mul(pr, pe_s, cos_sh_sb)
                    t1 = qup.tile([P, SSH], bf, tag="pet")
                    nc.vector.tensor_mul(t1, xs, sin_sh_sb)
                    nc.vector.tensor_add(pr, pr, t1)
                    bq = b_q1 if half == 0 else b_q2
                    for db in (0, GROUP):     # duplicate for both batches
                        base = (owner + db) * QSH
                        nc.sync.dma_start(out=bq[base:base + P, :],
                                          in_=evs[0])
                        nc.sync.dma_start(
                            out=bq[base + P:base + P + ROPE, :],
                            in_=pr[0:ROPE, :])
                        nc.sync.dma_start(
                            out=bq[base + QK_HD:base + QK_HD + P, :],
                            in_=evs[1])
                        nc.sync.dma_start(
                            out=bq[base + QK_HD + P:base + QSH, :],
                            in_=pr[ROPE:, :])
                    if pi == 3:
                        nc.gpsimd.collective_compute(
                            "AllToAll", mybir.AluOpType.bypass,
                            replica_groups=RG8, ins=[b_q1[:]],
                            outs=[o_q1[:]])
                        qt_assemble(0, o_q1)
                    if pi == 7:
                        nc.gpsimd.collective_compute(
                            "AllToAll", mybir.AluOpType.bypass,
                            replica_groups=RG8, ins=[b_q2[:]],
                            outs=[o_q2[:]])
                        qt_assemble(1, o_q2)

                # ---- kv up-proj (prescale by broadcast a_kv) ----
                for r in range(NCH):
                    bps = bc_ps.tile([P, SSH], f32, tag="bc")
                    nc.tensor.matmul(bps, ones1, akv_row[:, r, :])
                    akv = qup.tile([P, SSH], bf, tag="akv")
                    nc.vector.tensor_copy(akv, bps)
                    for k in range(NKVM):
                        nc.vector.tensor_mul(kv_lat[:, r, k, :],
                                             kv_lat[:, r, k, :], akv)
                for c in range(NCH):
                    for h in range(HPC):
                        ps = up_ps.tile([P, SSH], f32, tag="qps")
                        for k in range(NKVM):
                            nc.tensor.matmul(
                                ps, wkvb[:, k, h * NOPE:(h + 1) * NOPE],
                                kv_lat[:, c, k, :], start=(k == 0),
                                stop=(k == NKVM - 1))
                        nc.vector.tensor_copy(
                            kt_nope[h][:, c * SSH:(c + 1) * SSH], ps)
                for sb in range(S // P):
                    c, part = sb // 4, sb % 4
                    ps = up_ps.tile([P, HPC * V_DIM], f32, tag="qps")
                    for k in range(NKVM):
                        nc.tensor.matmul(
                            ps, kv_lat[:, c, k, part * P:(part + 1) * P],
                            wkvb[:, k, HPC * NOPE:], start=(k == 0),
                            stop=(k == NKVM - 1))
                    nc.vector.tensor_copy(v_all[:, sb, :], ps)


            up_lat_cm.__exit__(None, None, None)

            # ===== attention + per-head A2A + interleaved wo =====
            with (
                tc.tile_pool(name="at", bufs=3) as at,
                tc.tile_pool(name="atp", bufs=16) as atp,
                tc.tile_pool(name="at_ps", bufs=2) as at_psum_sb,
                tc.tile_pool(name="at_rl", bufs=2) as at_rl,
                tc.tile_pool(name="wo_rhs", bufs=3) as wo_rhs,
                tc.tile_pool(name="wo_acc", bufs=1) as wo_acc,
                tc.tile_pool(name="wo_w", bufs=3) as wo_w,
                tc.tile_pool(name="wo_ev", bufs=3) as wo_ev,
                tc.tile_pool(name="st_ps", bufs=2, space="PSUM") as st_ps,
                tc.tile_pool(name="ot_ps", bufs=2, space="PSUM") as ot_ps,
                tc.tile_pool(name="l_ps", bufs=1, space="PSUM") as l_ps,
                tc.tile_pool(name="rb_ps", bufs=1, space="PSUM") as rb_ps,
                tc.tile_pool(name="wo_ps", bufs=2, space="PSUM") as wo_ps,
            ):
                acc = wo_acc.tile([P, NKT, SSH], f32)

                def attention_head(h):
                    pend = []        # [(pj, off, first, last, ot, j), ...]
                    pend_ev = None   # (ot, P_sum, qc)

                    def flush2():
                        while pend:
                            pj, off, first, last, ot, j = pend.pop(0)
                            nc.tensor.matmul(
                                ot[:, off:],
                                v_all[:, j, h * V_DIM:(h + 1) * V_DIM],
                                pj[:, off:], start=first, stop=last)

                    def evict():
                        nonlocal pend_ev
                        if pend_ev is None:
                            return
                        ot, psum, qc = pend_ev
                        lt = l_ps.tile([1, SSH], f32, tag="l", name="lt")
                        nc.tensor.matmul(lt, ones_b, psum)
                        rinv = at_rl.tile([1, SSH], f32, tag="ri",
                                          name="rinv")
                        nc.vector.reciprocal_approx_fast(rinv, lt)
                        rinvb = at_rl.tile([1, SSH], bf, tag="rib",
                                           name="rinvb")
                        nc.vector.tensor_copy(rinvb, rinv)
                        rbp = rb_ps.tile([P, SSH], f32, tag="rb",
                                         name="rbp")
                        nc.tensor.matmul(rbp, ones1, rinvb)
                        rlb = at_rl.tile([P, SSH], bf, tag="rlb",
                                         name="rlb")
                        nc.vector.tensor_copy(rlb, rbp)
                        ev = at.tile([P, SSH], bf, tag="oev", name="oev")
                        nc.vector.tensor_mul(ev, ot, rlb)
                        # A2A shard for rank b*4+qc of each batch b
                        nc.sync.dma_start(
                            out=b_o[h][qc * P:(qc + 1) * P, :], in_=ev)
                        nc.sync.dma_start(
                            out=b_o[h][(GROUP + qc) * P:
                                       (GROUP + qc + 1) * P, :], in_=ev)
                        pend_ev = None

                    for qc in range(NCH):
                        nj = qc * 4 + 4
                        ot = ot_ps.tile([P, SSH], f32, tag="ot", name="ot")
                        psum = at_psum_sb.tile([P, SSH], bf, tag="ps",
                                               name="psum")
                        for j in range(nj):
                            d = j - qc * 4
                            off = max(0, d) * P
                            st = st_ps.tile([P, SSH], f32, tag="st",
                                            name="st")
                            nc.tensor.matmul(
                                st[:, off:],
                                kt_nope[h][:, j * P:(j + 1) * P],
                                qt_nope[h][:, qc * SSH + off:(qc + 1) * SSH],
                                start=True, stop=False)
                            lo = (h % 2) * ROPE
                            nc.tensor.matmul(
                                st[:, off:],
                                kpe_dup[lo:lo + ROPE, j // 4,
                                        (j % 4) * P:(j % 4 + 1) * P],
                                qt_pe[h // 2][lo:lo + ROPE,
                                              qc * SSH + off:(qc + 1) * SSH],
                                start=False, stop=True)
                            if j == 1:
                                evict()  # previous qc, off the exp path
                            pj = atp.tile([P, SSH], bf, tag="p", name="pj")
                            nc.scalar.activation(
                                pj[:, off:], st[:, off:],
                                mybir.ActivationFunctionType.Exp)
                            if d >= 0:
                                # multiplicative causal mask (vector)
                                nc.vector.tensor_mul(
                                    pj[:, off:], pj[:, off:],
                                    mask_sb[:, 0:SSH - off])
                            if j == 0:
                                nc.vector.tensor_copy(psum, pj)
                            else:
                                nc.vector.tensor_add(psum[:, off:],
                                                     psum[:, off:],
                                                     pj[:, off:])
                            pend.append((pj, off, j == 0, j == nj - 1,
                                         ot, j))
                        flush2()
                        pend_ev = (ot, psum, qc)
                    evict()
                    nc.gpsimd.collective_compute(
                        "AllToAll", mybir.AluOpType.bypass,
                        replica_groups=RG8, ins=[b_o[h][:]],
                        outs=[o_r[h][:]])
                    # wo weights prefetch on sync (no deps); rhs on gpsimd
                    # right behind the blocking A2A instruction so the
                    # scalar queue (exp) is never convoyed by these waits
                    wslab = wo_w.tile([P, GROUP, DIM], bf, tag="woslab",
                                      name="wslab")
                    nc.sync.dma_start(out=wslab, in_=woP[:, h])
                    rhs = wo_rhs.tile([P, GROUP, SSH], bf, tag="rhs",
                                      name="rhs")
                    for k in range(GROUP):
                        nc.gpsimd.dma_start(
                            out=rhs[:, k, :],
                            in_=o_r[h][bass.ds(rb_base + k * P, P), :])
                    return rhs, wslab

                def wo_pass(h, rhs, wslab):
                    for m in range(NKT):
                        ps = wo_ps.tile([P, SSH], f32, tag="wops",
                                        name="wops")
                        for k in range(GROUP):
                            nc.tensor.matmul(
                                ps, wslab[:, k, m * P:(m + 1) * P],
                                rhs[:, k, :], start=(k == 0),
                                stop=(k == GROUP - 1))
                        if h == 0:
                            nc.vector.tensor_copy(acc[:, m, :], ps)
                        elif h < HPC - 1:
                            nc.vector.tensor_add(acc[:, m, :], ps,
                                                 acc[:, m, :])
                        else:
                            ev = wo_ev.tile([P, SSH], f32, tag="woev",
                                            name="woev")
                            nc.vector.tensor_add(ev, ps, acc[:, m, :])
                            nc.sync.dma_start(out=outT[m * P:(m + 1) * P, :],
                                              in_=ev)

                heads_rhs = {}
                for h in range(HPC):
                    heads_rhs[h] = attention_head(h)
                    if h >= 1:
                        wo_pass(h - 1, *heads_rhs[h - 1])
                wo_pass(HPC - 1, *heads_rhs[HPC - 1])

    nc.compile()
    return nc


def _prep_inputs(x, freqs_cos, freqs_sin, wq_a, q_norm_w, wq_b, wkv_a,
                 kv_norm_w, wkv_b, wo):
    x = np.asarray(x, np.float32)
    freqs_cos = np.asarray(freqs_cos, np.float32)
    freqs_sin = np.asarray(freqs_sin, np.float32)
    wq_a = np.asarray(wq_a, np.float32)
    q_norm_w = np.asarray(q_norm_w, np.float32)
    wq_b = np.asarray(wq_b, np.float32)
    wkv_a = np.asarray(wkv_a, np.float32)
    kv_norm_w = np.asarray(kv_norm_w, np.float32)
    wkv_b = np.asarray(wkv_b, np.float32)
    wo = np.asarray(wo, np.float32)

    wqaT = np.ascontiguousarray(wq_a.T)          # [DIM, Q_LORA]
    wkvaT = np.ascontiguousarray(wkv_a.T)        # [DIM, KV_LORA+ROPE]
    wqaP = np.ascontiguousarray(
        wqaT.reshape(NKT, P, NQM, P).transpose(1, 2, 0, 3)).astype(BF16)
    wkvaP = np.ascontiguousarray(
        wkvaT[:, :KV_LORA].reshape(NKT, P, NKVM, P)
        .transpose(1, 2, 0, 3)).astype(BF16)
    wkpeP = np.ascontiguousarray(
        wkvaT[:, KV_LORA:].reshape(NKT, P, ROPE)
        .transpose(1, 0, 2)).astype(BF16)

    wqb_eff = (wq_b * q_norm_w[None, :]) * SCALE
    wqb_eff = wqb_eff.reshape(N_HEADS, QK_HD, Q_LORA)
    wkvb_eff = wkv_b * kv_norm_w[None, :]
    wkvb_eff = wkvb_eff.reshape(N_HEADS, NOPE + V_DIM, KV_LORA)

    cosT = np.tile(np.repeat(freqs_cos.T, 2, axis=0), (2, 1))  # [128, S]
    sinT = np.tile(np.repeat(freqs_sin.T, 2, axis=0), (2, 1))

    perm64_ = np.zeros((ROPE, ROPE), np.float32)
    for i in range(ROPE // 2):
        perm64_[2 * i + 1, 2 * i] = -1.0  # out[2i]   = -x[2i+1]
        perm64_[2 * i, 2 * i + 1] = 1.0   # out[2i+1] =  x[2i]
    perm = np.zeros((P, P), np.float32)
    perm[:ROPE, :ROPE] = perm64_
    perm[ROPE:, ROPE:] = perm64_
    r = np.arange(P)
    # multiplicative causal mask for diagonal score blocks: [kv r, q t]
    wmask = np.ones((P, SSH), np.float32)
    wmask[:, :P] = np.where(r[:, None] <= r[None, :], 1.0, 0.0)

    # wo.T rows regrouped so pass h contracts head k*4+h for k=0..3
    woT4 = wo.T.reshape(N_HEADS // 4, 4, V_DIM, DIM)  # [k, h, p, D]
    woP = np.ascontiguousarray(woT4.transpose(2, 1, 0, 3)).astype(BF16)

    # q-up weight columns pair-major: pair pi = half*4 + owner covers
    # heads A = owner*4 + half*2, B = A+1; block = [nopeA|nopeB|peA|peB]
    wqb_cols = np.zeros((N_HEADS * QK_HD, Q_LORA), np.float32)
    for pi in range(N_HEADS // 2):
        half, owner = pi // 4, pi % 4
        ha = owner * HPC + half * 2
        base = pi * QSH
        wqb_cols[base:base + NOPE] = wqb_eff[ha, :NOPE]
        wqb_cols[base + NOPE:base + 2 * NOPE] = wqb_eff[ha + 1, :NOPE]
        wqb_cols[base + 2 * NOPE:base + 2 * NOPE + ROPE] = wqb_eff[ha, NOPE:]
        wqb_cols[base + 2 * NOPE + ROPE:base + QSH] = wqb_eff[ha + 1, NOPE:]
    wqbT = wqb_cols.T                                 # [Q_LORA, 16*192]
    wqbP = np.ascontiguousarray(
        wqbT.reshape(NQM, P, N_HEADS * QK_HD).transpose(1, 0, 2)).astype(BF16)

    in_maps = []
    for c in range(NCORES):
        b, g = c // GROUP, c % GROUP
        heads = slice(g * HPC, (g + 1) * HPC)
        xTc = np.ascontiguousarray(x[b].T[:, g * SSH:(g + 1) * SSH])
        xPc = np.ascontiguousarray(
            xTc.reshape(NKT, P, SSH).transpose(1, 0, 2)).astype(BF16)
        wkvbT = np.concatenate(
            [wkvb_eff[heads, :NOPE].reshape(HPC * NOPE, KV_LORA),
             wkvb_eff[heads, NOPE:].reshape(HPC * V_DIM, KV_LORA)],
            axis=0).T                                  # [KV_LORA, 1024]
        wkvbP = np.ascontiguousarray(
            wkvbT.reshape(NKVM, P, HPC * (NOPE + V_DIM))
            .transpose(1, 0, 2)).astype(BF16)
        in_maps.append({
            "xP": xPc,
            "wqaP": wqaP,
            "wkvaP": wkvaP,
            "wkpeP": wkpeP,
            "wqbP": wqbP,
            "wkvbP": wkvbP,
            "woP": woP,
            "cos_sh": np.ascontiguousarray(
                cosT[:, g * SSH:(g + 1) * SSH]).astype(BF16),
            "sin_sh": np.ascontiguousarray(
                sinT[:, g * SSH:(g + 1) * SSH]).astype(BF16),
            "perm64": perm.astype(BF16),
            "wmask": wmask.astype(BF16),
            "cfg": np.array([[b * GROUP * P, b * GROUP * QSH]], np.int32),
        })
    return in_maps


def _run(inputs, trace=False, **kw):
    if "nc" not in _cache:
        _cache["nc"] = _build()
    nc = _cache["nc"]
    in_maps = _prep_inputs(**inputs)
    res = run_bass_kernel_spmd(nc, in_maps, core_ids=list(range(NCORES)),
                               trace=trace, **kw)
    out = np.empty((B, S, DIM), np.float32)
    for c in range(NCORES):
        b, g = c // GROUP, c % GROUP
        out[b, g * SSH:(g + 1) * SSH, :] = res.results[c]["out"].T
    return out, res


def kernel(**inputs):
    out, _ = _run(inputs)
    return out
